# revision 1
# baseline (speedup 1.0000x reference)
"""LocalSelfAttention (window=7) Trainium2 Bass kernel.

Full inputs in, full output out. Sharding: 8 cores = batch(4) x seq-half(2),
each core handles 1024 tokens with a 3-token zero-padded halo on xs.

Math notes (exact rewrites of the reference):
- reference projects zero-PADDED xs patches, so out-of-range taps have
  k = b_ks, v = b_vs. Softmax over taps is invariant to the per-(t,h)
  constant q . b_ks, so the K bias drops entirely (padded taps then score 0,
  matching zero-padded halo @ w_ks with no bias).
- softmax weights sum to 1, so the V bias contributes exactly b_vs to o;
  it is folded on the host into the residual: xq32 = x + b_vs @ w_fc + b_fc.

Pipeline per core (feature-major activations, transposed on the HOST):
- QT feature-major via matmul(lhsT=weight tile, rhs=xT); KT evicted into a
  BLOCK-DIAGONAL layout KTz[ec] = [128, 2, TH2] (head even in rows 0:64 of
  slot 0, head odd in rows 64:128 of slot 1, zeros elsewhere) so one N=256
  matmul computes both heads' windowed scores; V token-major.
- attention in 9 chunks of 122 tokens (window 122+6=128), TWO head pairs
  (4 heads) per iteration: 2 score matmuls land in the two banks of one
  PSUM tile (122, 1024), band-masked softmax with 4-head-wide DVE/ACT ops
  (exp in bf16), 4 PE-transposes of the prob slots, 4 PV matmuls into one
  PSUM tile evicted by a single strided ACT into a unified OT tile.
- V projection chunks and FC(+residual+layernorm) chunks are emitted
  INSIDE the attention loop as their dependencies complete, so the PE
  queue never sits behind a phase barrier; PSUM pools are phase-scoped
  (projection pool released before the attention pools are created).
- FC residual add reads PSUM directly (a fused PSUM-source
  tensor_tensor_reduce crashes the exec unit, a plain add is fine).
"""

import sys

for _p in ("/opt/trn_rl_repo",):
    if _p not in sys.path:
        sys.path.insert(0, _p)

import numpy as np
import ml_dtypes

BF16 = ml_dtypes.bfloat16

H, DK, DV, D = 16, 64, 64, 1024
NEI = 3
TEMP = 8.0
EPS = 1e-5
B, S = 4, 2048
NCORES = 8
T = (B * S) // NCORES          # 1024 tokens per core
TH = T + 2 * NEI               # 1030 halo tokens
P = 128
NT = T // P                    # 8 fc-phase token chunks
ND = D // P                    # 8 feature chunks
CL = 122                       # attention chunk length (window 122+6=128)
CST = [122 * i for i in range(8)] + [902]          # chunk starts
TH2 = 1056                     # padded halo width (window reads up to 1056)
NEG = -30000.0

_CACHE = {}


def _build_program(apply_affine: bool):
    import concourse.bacc as bacc
    import concourse.tile as tile
    from concourse import mybir
    from contextlib import ExitStack

    f32 = mybir.dt.float32
    bf16 = mybir.dt.bfloat16
    Alu = mybir.AluOpType
    Act = mybir.ActivationFunctionType

    nc = bacc.Bacc(
        "TRN2", target_bir_lowering=False, debug=False, enable_asserts=False
    )

    def din(name, shape, dt_):
        return nc.dram_tensor(name, shape, dt_, kind="ExternalInput").ap()

    xq32 = din("xq32", (T, D), f32)      # residual + folded FC bias (f32)
    xqT = din("xqT", (D, T), bf16)       # x^T (host-transposed)
    xsT = din("xsT", (D, TH), bf16)      # xs^T with halo (host-transposed)
    wq = din("wq", (D, D), bf16)
    wk = din("wk", (D, D), bf16)
    wv = din("wv", (D, D), bf16)
    wf = din("wf", (D, D), bf16)
    bq = din("bq", (P, ND), f32)         # b_qs laid out [p, ec]
    msk = din("msk", (CL, 4 * P), bf16)  # multiplicative band mask 0 / 1
    idn = din("idn", (P, P), bf16)       # identity for PE transpose
    if apply_affine:
        lng = din("lng", (1, D), f32)
        lnb = din("lnb", (1, D), f32)
    yo = nc.dram_tensor("yo", (T, D), f32, kind="ExternalOutput").ap()

    with tile.TileContext(nc) as tc, ExitStack() as ctx:
        consts = ctx.enter_context(tc.tile_pool(name="consts", bufs=1))
        big = ctx.enter_context(tc.tile_pool(name="big", bufs=1))
        wpool = ctx.enter_context(tc.tile_pool(name="wpool", bufs=2))
        xrpool = ctx.enter_context(tc.tile_pool(name="xrpool", bufs=3))
        work = ctx.enter_context(tc.tile_pool(name="work", bufs=3))
        lnpool = ctx.enter_context(tc.tile_pool(name="lnpool", bufs=2))
        small = ctx.enter_context(tc.tile_pool(name="small", bufs=4))
        # projection-phase PSUM pool: released before attention so the
        # attention/FC pools (psS+psT+psO+psF = 8 banks, created after the
        # release) can reuse its banks
        psP = tc.alloc_tile_pool(name="psP", bufs=3, space="PSUM")

        # ---- first Q-projection operands ahead of everything else ----
        wq_t = []
        xT_t = []
        wt0 = wpool.tile([P, D], bf16, tag="w0", name="w_q0")
        nc.sync.dma_start(out=wt0, in_=wq[0:P, :])
        wq_t.append(wt0)
        t10 = big.tile([P, T], bf16, tag="xT0", name="xT0")
        nc.sync.dma_start(out=t10, in_=xqT[0:P, :])
        xT_t.append(t10)

        # ---- constants ----
        msk_sb = consts.tile([CL, 4 * P], bf16, tag="msk")
        nc.sync.dma_start(out=msk_sb, in_=msk)
        idn_sb = consts.tile([P, P], bf16, tag="idn")
        nc.sync.dma_start(out=idn_sb, in_=idn)
        bq_sb = consts.tile([P, ND], f32, tag="bq")
        nc.sync.dma_start(out=bq_sb, in_=bq)
        eps_sb = consts.tile([P, 1], f32, tag="eps")
        nc.vector.memset(eps_sb, EPS)
        one_u32 = consts.tile([P, 1], mybir.dt.uint32, tag="one32")
        nc.vector.memset(one_u32, 1)
        magic_sb = consts.tile([P, 1], mybir.dt.uint32, tag="magic")
        nc.vector.memset(magic_sb, 0x5f3759df)
        if apply_affine:
            import concourse.bass as bass

            g_bc = consts.tile([P, D], f32, tag="g_bc")
            b_bc = consts.tile([P, D], f32, tag="b_bc")
            nc.sync.dma_start(
                out=g_bc,
                in_=bass.AP(tensor=lng.tensor, offset=lng.offset,
                            ap=[[0, P]] + list(lng.ap[1:])),
            )
            nc.sync.dma_start(
                out=b_bc,
                in_=bass.AP(tensor=lnb.tensor, offset=lnb.offset,
                            ap=[[0, P]] + list(lnb.ap[1:])),
            )

        def load_w(wap, tagp):
            tiles = []
            for dc in range(ND):
                wt = wpool.tile([P, D], bf16, tag=f"w{dc}", name=f"w_{tagp}{dc}")
                nc.sync.dma_start(out=wt, in_=wap[dc * P:(dc + 1) * P, :])
                tiles.append(wt)
            return tiles

        # ---- activation loads interleaved with the weights that consume
        # them, so the Q projection can start as soon as possible ----
        xsT_t = []
        wk_t = []
        for dc in range(1, ND):
            wt = wpool.tile([P, D], bf16, tag=f"w{dc}", name=f"w_q{dc}")
            nc.sync.dma_start(out=wt, in_=wq[dc * P:(dc + 1) * P, :])
            wq_t.append(wt)
            t1 = big.tile([P, T], bf16, tag=f"xT{dc}", name=f"xT{dc}")
            nc.sync.dma_start(out=t1, in_=xqT[dc * P:(dc + 1) * P, :])
            xT_t.append(t1)
        for dc in range(ND):
            wt = wpool.tile([P, D], bf16, tag=f"w{dc}", name=f"w_k{dc}")
            nc.sync.dma_start(out=wt, in_=wk[dc * P:(dc + 1) * P, :])
            wk_t.append(wt)
            t2 = big.tile([P, TH2], bf16, tag=f"xsT{dc}", name=f"xsT{dc}")
            nc.sync.dma_start(out=t2[:, 0:TH], in_=xsT[dc * P:(dc + 1) * P, :])
            nc.vector.memset(t2[:, TH:TH2], 0.0)
            xsT_t.append(t2)

        # ---- QT projection: (e, t) feature-major, bias via ACT evict ----
        QT = [big.tile([P, T], bf16, tag=f"QT{ec}", name=f"QT{ec}")
              for ec in range(ND)]
        for ec in range(ND):
            psa = psP.tile([P, 512], f32, tag="psA", name="ps_qa")
            psb = psP.tile([P, 512], f32, tag="psB", name="ps_qb")
            for dc in range(ND):
                lt = wq_t[dc][:, ec * P:(ec + 1) * P]
                nc.tensor.matmul(psa, lhsT=lt, rhs=xT_t[dc][:, 0:512],
                                 start=(dc == 0), stop=(dc == ND - 1))
                nc.tensor.matmul(psb, lhsT=lt, rhs=xT_t[dc][:, 512:1024],
                                 start=(dc == 0), stop=(dc == ND - 1))
            nc.scalar.activation(out=QT[ec][:, 0:512], in_=psa,
                                 func=Act.Identity,
                                 bias=bq_sb[:, ec:ec + 1], scale=1.0)
            nc.scalar.activation(out=QT[ec][:, 512:1024], in_=psb,
                                 func=Act.Identity,
                                 bias=bq_sb[:, ec:ec + 1], scale=1.0)

        # ---- KT projection: block-diagonal (e, slot, t_halo), no bias ----
        # KTz[ec][0:64, 0, :] = K head 2ec, KTz[ec][64:128, 1, :] = K head
        # 2ec+1, zeros elsewhere, so scores for the pair are ONE N=256 matmul.
        KTz = [big.tile([P, 2 * TH2], bf16, tag=f"KTz{ec}", name=f"KTz{ec}")
               for ec in range(ND)]
        for ec in range(ND):
            nc.gpsimd.memset(KTz[ec][64:128, 0:TH2], 0.0)
            nc.gpsimd.memset(KTz[ec][0:64, TH2:2 * TH2], 0.0)
        for ec in range(ND):
            psa = psP.tile([P, 512], f32, tag="psA", name="ps_ka")
            psb = psP.tile([P, 512], f32, tag="psB", name="ps_kb")
            for dc in range(ND):
                lt = wk_t[dc][:, ec * P:(ec + 1) * P]
                nc.tensor.matmul(psa, lhsT=lt, rhs=xsT_t[dc][:, 0:512],
                                 start=(dc == 0), stop=(dc == ND - 1))
                nc.tensor.matmul(psb, lhsT=lt, rhs=xsT_t[dc][:, 512:1024],
                                 start=(dc == 0), stop=(dc == ND - 1))
            nc.scalar.activation(out=KTz[ec][0:64, 0:512], in_=psa[0:64, :],
                                 func=Act.Copy)
            nc.scalar.activation(out=KTz[ec][64:128, TH2:TH2 + 512],
                                 in_=psa[64:128, :], func=Act.Copy)
            nc.scalar.activation(out=KTz[ec][0:64, 512:1024],
                                 in_=psb[0:64, :], func=Act.Copy)
            nc.scalar.activation(out=KTz[ec][64:128, TH2 + 512:TH2 + 1024],
                                 in_=psb[64:128, :], func=Act.Copy)
        for ec in range(ND):  # halo tail (incl zero padding)
            pst = psP.tile([P, TH2 - T], f32, tag="psA", name="ps_kt")
            for dc in range(ND):
                nc.tensor.matmul(pst,
                                 lhsT=wk_t[dc][:, ec * P:(ec + 1) * P],
                                 rhs=xsT_t[dc][:, T:TH2],
                                 start=(dc == 0), stop=(dc == ND - 1))
            nc.vector.tensor_copy(KTz[ec][0:64, T:TH2], pst[0:64, :])
            nc.vector.tensor_copy(KTz[ec][64:128, TH2 + T:2 * TH2],
                                  pst[64:128, :])

        # ---- V projection: token-major (halo-rows, e); 11 chunk tiles,
        # emitted lazily (3-chunk prologue, then interleaved with attention
        # so attention does not wait for the whole V projection) ----
        wv_t = load_w(wv, "v")
        # prefetch FC weights during attention
        wf_t = load_w(wf, "f")
        psP.release()
        psV = ctx.enter_context(tc.tile_pool(name="psV", bufs=1, space="PSUM"))
        psS = ctx.enter_context(tc.tile_pool(name="psS", bufs=1, space="PSUM"))
        psT = ctx.enter_context(tc.tile_pool(name="psT", bufs=1, space="PSUM"))
        psO = ctx.enter_context(tc.tile_pool(name="psO", bufs=1, space="PSUM"))
        psF = ctx.enter_context(tc.tile_pool(name="psF", bufs=1, space="PSUM"))

        # ---- windowed attention: chunks of 96, TWO head pairs / iter ----
        # FC chunks are emitted INSIDE the attention loop once the OT
        # columns they consume are complete, with one chunk of slack so a
        # late OT eviction can't head-of-line-block the PE queue; the last
        # 256 tokens use 64-token chunks to shrink the serial tail.
        FC_AFTER = {2: ((0, P),), 3: ((P, P),), 4: ((2 * P, P),),
                    5: ((3 * P, P),), 6: ((4 * P, P),),
                    7: ((5 * P, P),), 8: ((6 * P, P), (7 * P, P))}
        FC_BEFORE = {}
        OTall = big.tile([P, ND * T], bf16, tag="OTall", name="OTall")
        OTv = OTall.rearrange("p (e t) -> p e t", e=ND)

        V = []

        def emit_v(ci):
            s = CST[ci]
            vt = big.tile([P, D], bf16, tag=f"V{ci}", name=f"V{ci}")
            psa = psV.tile([P, 512], f32, tag="psVa", name="ps_va")
            psb = psV.tile([P, 512], f32, tag="psVb", name="ps_vb")
            for dc in range(ND):
                lt = xsT_t[dc][:, s:s + P]
                nc.tensor.matmul(psa, lhsT=lt, rhs=wv_t[dc][:, 0:512],
                                 start=(dc == 0), stop=(dc == ND - 1))
                nc.tensor.matmul(psb, lhsT=lt, rhs=wv_t[dc][:, 512:1024],
                                 start=(dc == 0), stop=(dc == ND - 1))
            nc.scalar.activation(out=vt[:, 0:512], in_=psa, func=Act.Copy)
            nc.scalar.activation(out=vt[:, 512:1024], in_=psb, func=Act.Copy)
            V.append(vt)

        for ci in range(2):
            emit_v(ci)

        def emit_fc(t0, pl, act_sqrt=False):
            cs = slice(t0, t0 + pl)
            xr = xrpool.tile([P, D], f32, tag="xr", name="xr")
            nc.sync.dma_start(out=xr[0:pl, :], in_=xq32[cs, :])
            y_sb = lnpool.tile([P, D], f32, tag="ysb", name="y_sb")
            psa = psF.tile([P, 512], f32, tag="psFa", name="ps_fa")
            psb = psF.tile([P, 512], f32, tag="psFb", name="ps_fb")
            for ec in range(ND):
                nc.tensor.matmul(psa[0:pl, :], lhsT=OTv[:, ec, cs],
                                 rhs=wf_t[ec][:, 0:512],
                                 start=(ec == 0), stop=(ec == ND - 1))
            nc.vector.tensor_tensor(y_sb[0:pl, 0:512], psa[0:pl, :],
                                    xr[0:pl, 0:512], Alu.add)
            for ec in range(ND):
                nc.tensor.matmul(psb[0:pl, :], lhsT=OTv[:, ec, cs],
                                 rhs=wf_t[ec][:, 512:1024],
                                 start=(ec == 0), stop=(ec == ND - 1))
            nc.vector.tensor_tensor(y_sb[0:pl, 512:1024], psb[0:pl, :],
                                    xr[0:pl, 512:1024], Alu.add)
            ysum = None
            for dcol in range(2):
                ds_ = slice(dcol * 512, (dcol + 1) * 512)
                new_sum = small.tile([P, 1], f32, tag=f"ysum{dcol}",
                                     name="ysum")
                nc.vector.tensor_reduce(
                    out=new_sum[0:pl, :], in_=y_sb[0:pl, ds_],
                    axis=mybir.AxisListType.X, op=Alu.add,
                )
                if ysum is not None:
                    nsum2 = small.tile([P, 1], f32, tag="nsum2", name="nsum2")
                    nc.vector.tensor_add(nsum2[0:pl, :], new_sum[0:pl, :],
                                         ysum[0:pl, :])
                    new_sum = nsum2
                ysum = new_sum
            sqs = []
            for dcol in range(2):
                ds_ = slice(dcol * 512, (dcol + 1) * 512)
                ysq = lnpool.tile([P, 512], f32, tag="ysq", name="ysq")
                sq = small.tile([P, 1], f32, tag=f"sq{dcol}", name="sq")
                nc.scalar.activation(out=ysq[0:pl, :], in_=y_sb[0:pl, ds_],
                                     func=Act.Square, accum_out=sq[0:pl, :])
                sqs.append(sq)
            ssum = small.tile([P, 1], f32, tag="ssum", name="ssum")
            nc.vector.tensor_add(ssum[0:pl, :], sqs[0][0:pl, :],
                                 sqs[1][0:pl, :])
            mean = small.tile([P, 1], f32, tag="mean", name="mean")
            nc.vector.tensor_scalar_mul(mean[0:pl, :], ysum[0:pl, :], 1.0 / D)
            msq = small.tile([P, 1], f32, tag="msq", name="msq")
            nc.vector.tensor_mul(msq[0:pl, :], mean[0:pl, :], mean[0:pl, :])
            var = small.tile([P, 1], f32, tag="var", name="var")
            nc.vector.scalar_tensor_tensor(
                out=var[0:pl, :], in0=ssum[0:pl, :], scalar=1.0 / D,
                in1=msq[0:pl, :], op0=Alu.mult, op1=Alu.subtract,
            )
            # rsqrt(var+eps) entirely on DVE (bit-trick seed + 2 Newton
            # steps, ~4e-6 rel err): an ACT Sqrt here would force two
            # activation-table reloads per FC chunk (EXP<->SQRT thrash).
            # The LAST chunk runs after the final EXP, so ACT Sqrt is free.
            if act_sqrt:
                std = small.tile([P, 1], f32, tag="std", name="std")
                nc.scalar.activation(out=std[0:pl, :], in_=var[0:pl, :],
                                     func=Act.Sqrt, bias=eps_sb[0:pl, :])
                rstd = small.tile([P, 1], f32, tag="rstd", name="rstd")
                nc.vector.reciprocal(rstd[0:pl, :], std[0:pl, :])
            else:
                veps = small.tile([P, 1], f32, tag="veps", name="veps")
                nc.vector.tensor_scalar_add(veps[0:pl, :], var[0:pl, :], EPS)
                sh = small.tile([P, 1], mybir.dt.uint32, tag="sh", name="sh")
                nc.vector.tensor_tensor(
                    sh[0:pl, :], veps.bitcast(mybir.dt.uint32)[0:pl, :],
                    one_u32[0:pl, :], Alu.logical_shift_right)
                rstd = small.tile([P, 1], f32, tag="rstd", name="rstd")
                nc.vector.tensor_tensor(
                    rstd.bitcast(mybir.dt.uint32)[0:pl, :],
                    magic_sb[0:pl, :], sh[0:pl, :], Alu.subtract)
                for _nr in range(2):
                    t1n = small.tile([P, 1], f32, tag="t1n", name="t1n")
                    nc.vector.tensor_mul(t1n[0:pl, :], rstd[0:pl, :],
                                         rstd[0:pl, :])
                    nc.vector.tensor_mul(t1n[0:pl, :], t1n[0:pl, :],
                                         veps[0:pl, :])
                    nc.vector.tensor_scalar(
                        out=t1n[0:pl, :], in0=t1n[0:pl, :],
                        scalar1=-0.5, scalar2=1.5, op0=Alu.mult, op1=Alu.add)
                    nc.vector.tensor_mul(rstd[0:pl, :], rstd[0:pl, :],
                                         t1n[0:pl, :])
            bact = small.tile([P, 1], f32, tag="bact", name="bact")
            nc.vector.scalar_tensor_tensor(
                out=bact[0:pl, :], in0=mean[0:pl, :], scalar=-1.0,
                in1=rstd[0:pl, :], op0=Alu.mult, op1=Alu.mult,
            )
            out_sb = lnpool.tile([P, D], f32, tag="osb", name="out_sb")
            nc.scalar.activation(out=out_sb[0:pl, :], in_=y_sb[0:pl, :],
                                 func=Act.Identity,
                                 bias=bact[0:pl, :], scale=rstd[0:pl, :])
            if apply_affine:
                nc.vector.tensor_mul(out_sb[0:pl, :], out_sb[0:pl, :],
                                     g_bc[0:pl, :])
                nc.vector.tensor_add(out_sb[0:pl, :], out_sb[0:pl, :],
                                     b_bc[0:pl, :])
            nc.sync.dma_start(out=yo[cs, :], in_=out_sb[0:pl, :])

        for ci, s in enumerate(CST):
            for t0, pl in FC_BEFORE.get(ci, ()):
                emit_fc(t0, pl)
            for e2 in range(4):  # pairs (2*e2, 2*e2+1) -> heads 4*e2..4*e2+3
                ecA, ecB = 2 * e2, 2 * e2 + 1
                # one N=256 block-diag score matmul per pair; the two pairs
                # go to the two BANKS of one psum tile
                s2 = psS.tile([CL, 1024], f32, tag="psS", name="s2")
                kzA = KTz[ecA].rearrange("p (s t) -> p s t", s=2)
                kzB = KTz[ecB].rearrange("p (s t) -> p s t", s=2)
                nc.tensor.matmul(
                    s2[:, 0:256],
                    lhsT=QT[ecA][:, s:s + CL],
                    rhs=kzA[:, :, s:s + P],
                    start=True, stop=True,
                )
                nc.tensor.matmul(
                    s2[:, 512:768],
                    lhsT=QT[ecB][:, s:s + CL],
                    rhs=kzB[:, :, s:s + P],
                    start=True, stop=True,
                )
                sv = s2.rearrange("p (b c) -> p b c", b=2)[:, :, 0:256]
                pe2 = work.tile([CL, 4 * P], bf16, tag="pe2", name="pe2")
                nc.scalar.activation(out=pe2.rearrange("p (b c) -> p b c", b=2),
                                     in_=sv, func=Act.Exp, scale=1.0 / TEMP)
                pet = work.tile([CL, 4 * P], bf16, tag="pet", name="pet")
                nc.vector.tensor_tensor(pet, pe2, msk_sb, Alu.mult)
                rs2 = small.tile([CL, 4], f32, tag="rs2", name="rs2")
                nc.vector.tensor_reduce(
                    out=rs2,
                    in_=pet.rearrange("a (h w) -> a h w", h=4),
                    axis=mybir.AxisListType.X, op=Alu.add,
                )
                rsr2 = small.tile([CL, 4], f32, tag="rsr2", name="rsr2")
                nc.vector.reciprocal(rsr2, rs2)
                pn2 = work.tile([CL, 4 * P], bf16, tag="pn2", name="pn2")
                nc.vector.tensor_tensor(
                    pn2.rearrange("a (h w) -> a h w", h=4),
                    pet.rearrange("a (h w) -> a h w", h=4),
                    rsr2[:, :, None].to_broadcast((CL, 4, P)),
                    Alu.mult,
                )
                pt_ps = psT.tile([P, 4 * CL], bf16, tag="psT", name="pt_ps")
                for h in range(4):
                    nc.tensor.transpose(pt_ps[:, h * CL:(h + 1) * CL],
                                        pn2[:, h * P:(h + 1) * P],
                                        idn_sb[0:CL, 0:CL])
                pt_sb = work.tile([P, 4 * CL], bf16, tag="ptsb", name="pt_sb")
                nc.scalar.activation(out=pt_sb, in_=pt_ps, func=Act.Copy)
                ot2 = psO.tile([P, 2 * CL], f32, tag="psO", name="ot2")
                for j, ec in enumerate((ecA, ecB)):
                    nc.tensor.matmul(
                        ot2[0:64, j * CL:(j + 1) * CL],
                        lhsT=V[ci][:, ec * P:ec * P + 64],
                        rhs=pt_sb[:, (2 * j) * CL:(2 * j + 1) * CL],
                        start=True, stop=True,
                    )
                    nc.tensor.matmul(
                        ot2[64:128, j * CL:(j + 1) * CL],
                        lhsT=V[ci][:, ec * P + 64:(ec + 1) * P],
                        rhs=pt_sb[:, (2 * j + 1) * CL:(2 * j + 2) * CL],
                        start=True, stop=True,
                    )
                nc.scalar.activation(
                    out=OTv[:, ecA:ecA + 2, s:s + CL],
                    in_=ot2.rearrange("p (e t) -> p e t", e=2),
                    func=Act.Copy)
            for t0, pl in FC_AFTER.get(ci, ()):
                emit_fc(t0, pl, act_sqrt=(t0 == 7 * P))
            if ci + 2 < len(CST):
                emit_v(ci + 2)

    nc.compile()
    return nc


def _get_program(apply_affine: bool):
    key = ("prog", apply_affine)
    if key not in _CACHE:
        _CACHE[key] = _build_program(apply_affine)
    return _CACHE[key]


def _host_prep(inputs):
    x = np.asarray(inputs["x"], np.float32)
    xs = np.asarray(inputs["xs"], np.float32)
    w_qs = np.asarray(inputs["w_qs"], np.float32)
    b_qs = np.asarray(inputs["b_qs"], np.float32)
    w_ks = np.asarray(inputs["w_ks"], np.float32)
    w_vs = np.asarray(inputs["w_vs"], np.float32)
    b_vs = np.asarray(inputs["b_vs"], np.float32)
    w_fc = np.asarray(inputs["w_fc"], np.float32)
    b_fc = np.asarray(inputs["b_fc"], np.float32)
    ln_g = np.asarray(inputs["ln_g"], np.float32)
    ln_b = np.asarray(inputs["ln_b"], np.float32)

    apply_affine = not (np.all(ln_g == 1.0) and np.all(ln_b == 0.0))

    bprime = (b_vs @ w_fc + b_fc).astype(np.float32)

    mask = np.zeros((CL, P), np.float32)
    for t in range(CL):
        mask[t, t:t + 2 * NEI + 1] = 1.0   # multiplicative band mask
    mask4 = np.concatenate([mask, mask, mask, mask], axis=1).astype(BF16)

    shared = {
        "wq": np.ascontiguousarray(w_qs.astype(BF16)),
        "wk": np.ascontiguousarray(w_ks.astype(BF16)),
        "wv": np.ascontiguousarray(w_vs.astype(BF16)),
        "wf": np.ascontiguousarray(w_fc.astype(BF16)),
        "bq": np.ascontiguousarray(b_qs.reshape(ND, P).T.astype(np.float32)),
        "msk": np.ascontiguousarray(mask4),
        "idn": np.eye(P, dtype=BF16),
    }
    if apply_affine:
        shared["lng"] = np.ascontiguousarray(ln_g.reshape(1, D))
        shared["lnb"] = np.ascontiguousarray(ln_b.reshape(1, D))

    in_maps = []
    half_n = S // 2  # 1024
    for core in range(NCORES):
        b, half = core // 2, core % 2
        t0 = half * half_n
        xq = x[b, t0:t0 + half_n]
        halo = np.zeros((TH, D), np.float32)
        lo = max(0, t0 - NEI)
        hi = min(S, t0 + half_n + NEI)
        halo[lo - (t0 - NEI):hi - (t0 - NEI)] = xs[b, lo:hi]
        m = dict(shared)
        m["xq32"] = np.ascontiguousarray(xq + bprime)
        m["xqT"] = np.ascontiguousarray(xq.T.astype(BF16))
        m["xsT"] = np.ascontiguousarray(halo.T.astype(BF16))
        in_maps.append(m)
    return in_maps, apply_affine


def _run(inputs, trace=False, trace_kwargs=None):
    from concourse.bass_utils import run_bass_kernel_spmd

    in_maps, apply_affine = _host_prep(inputs)
    nc = _get_program(apply_affine)
    res = run_bass_kernel_spmd(
        nc, in_maps, list(range(NCORES)),
        trace=trace, **(trace_kwargs or {})
    )
    y = np.empty((B, S, D), np.float32)
    half_n = S // 2
    for core in range(NCORES):
        b, half = core // 2, core % 2
        y[b, half * half_n:(half + 1) * half_n] = res.results[core]["yo"]
    return y, res


def kernel(**inputs):
    y, _ = _run(inputs)
    return y

